# revision 28
# baseline (speedup 1.0000x reference)
"""LMS adaptive filter (BaseFilter) on 8 TRN2 NeuronCores.

Algorithm per (batch b, frame f): 64-tap LMS over 416 sequential steps.
  e_t   = d[b, 256f + 32 + t] - sum_k w[k] * x[256f + t + k]
  w     = clip(w + MU * e_t * x[256f + t : +64], +-65535)
The clip is essential: mu*|x_win|^2 ~ 3.2 > 2 makes the recursion
unstable, so w rides the clip rails and the rails keep all float
implementations shadowing each other. f32 required (bf16 diverges).

Sharding: 4096 frames split 512/core (both batches on every core) ->
1024 independent sequences/core = 8 groups x 128 partitions.

The ENTIRE hot loop runs on the Vector engine as one in-order stream.
Rationale (measured): GpSimd shares an SBUF port with DVE's second
read port, so any Pool op overlapping a 2-source DVE op stalls it
("one fully blocks") -- this inflated every multi-engine variant ~25%;
Act costs ~280ns/op and its extra hop latency into the update loses
more than it saves.  On one engine the cost model is: independent ops
issue at (58 + FD/accel) cycles @0.96GHz; RAW-dependent ops wait the
predecessor's full duration (157 + FD/accel ns) + ~35ns ack.  Hence:
  8x dot  : STT64+accum -> NS8 slices (~146ns/pair, pipelined)
  2x e    : TT4 ET[:,t,4h:4h+4] = d_t + ns, interleaved INSIDE the
            dot stream (after dots 0-3 / 4-7) so accumulator-ack and
            TT-duration edges hide under later dots' issue slots
  2x TMP  : TT256 (mu x-window) * e-bcast (stride-0 AP)
  2x W'   : TT256 W + TMP          } halves: the h1 ops unblock the
  2x clip : TS256 min/max, 2x mode } chain while h2 issues in shadow
The Tile scheduler then software-pipelines next-step dots into the
clip shadow via subtile dependencies (~3.1us/step, ~44ns off the
issue-sum floor).  The last step's update is skipped (W is dead), and
input/output DMA descriptors are spread over the sync+scalar HWDGE
queues.  d_est is not computed on-chip: host does d_est = d - e
(exact, since the reference defines e = d - d_est).
"""

import numpy as np

HOP = 256
FRAMELEN = 512
K = 64
WD = 32
MU = 0.05
WMIN, WMAX = -65535.0, 65535.0
B = 2
F = 4096
NC = 8
F_LOC = F // NC              # 512 frames per core
S = (FRAMELEN - K) - WD      # 416 sequential steps
TSTART = (FRAMELEN - HOP) - WD  # 224: first step kept for frames >= 1
TAIL = S - TSTART            # 192 output elements per frame >= 1
SPAN = HOP * (F_LOC - 1) + FRAMELEN  # 131328: x/d elements per core shard
CORE_STRIDE = HOP * F_LOC    # 131072
OUT_LEN = (FRAMELEN - K) + (F - 1) * TAIL  # 786688

NBUF = 3                     # NS8/PROD buffer depth

_CACHE = {}


def _build():
    import concourse.bacc as bacc
    import concourse.tile as tile
    from concourse import mybir
    import concourse.bass as bass

    f32 = mybir.dt.float32
    AluOp = mybir.AluOpType

    nc = bacc.Bacc("TRN2", target_bir_lowering=False)
    x_in = nc.dram_tensor("x", [SPAN], f32, kind="ExternalInput")
    d_in = nc.dram_tensor("d", [B, SPAN], f32, kind="ExternalInput")
    # e only; d_est = d - e on host.  [b][f_local][j], j <-> t = TSTART + j
    out_e = nc.dram_tensor("out_e", [B, F_LOC, TAIL], f32,
                           kind="ExternalOutput")
    out_head = nc.dram_tensor("out_head", [B, TSTART], f32,
                              kind="ExternalOutput")

    with tile.TileContext(nc) as tc:
        with tc.tile_pool(name="p", bufs=1) as pool:
            XF = pool.tile([128, 4, FRAMELEN], f32)    # x frames (slab fg)
            DB = pool.tile([128, B, 4, S], f32)        # d at step offsets
            # all 8 groups' weights in one tile: W[:, g, :], g = 4b + fg
            WALL = [pool.tile([128, 8, K], f32, name=f"WALL{i}",
                              tag=f"wall{i}") for i in range(2)]
            TMP = [pool.tile([128, 8, K], f32, name=f"TMPALL{i}",
                             tag=f"tmpall{i}") for i in range(2)]
            # e history: ET[:, t, g]  (serves e-bcast reads AND output)
            ET = pool.tile([128, S, 8], f32, name="ET", tag="et")
            NS8 = [pool.tile([128, 8], f32, name=f"NS8_{i}", tag=f"n{i}")
                   for i in range(NBUF)]
            PROD = [pool.tile([128, 8, K], f32, name=f"PROD{i}",
                              tag=f"p{i}") for i in range(NBUF)]
            EOUT = pool.tile([128, 8, TAIL], f32, name="EOUT", tag="eout")

            # partition p, slab fg  ->  frame f_local = fg*128 + p
            # Head-optimized input loading: the first 64 steps need only
            # DB columns 0:64, so one wide head-chunk DMA per batch (all 4
            # slabs) lands fast; XF slabs split across both HWDGE queues;
            # DB tails (cols 64:416) deferred -- first needed ~200us in.
            HD = 64
            nc.vector.memset(WALL[0][:], 0.0)
            dba = DB[:]
            for b, eng in ((0, nc.scalar), (1, nc.scalar)):
                eng.dma_start(
                    bass.AP(tensor=dba.tensor,
                            offset=dba.offset + b * 4 * S,
                            ap=[list(dba.ap[0]), [S, 4], [1, HD]]),
                    bass.AP(tensor=d_in, offset=b * SPAN + WD,
                            ap=[[HOP, 128], [HOP * 128, 4], [1, HD]]),
                )
            for fg in range(4):
                eng = nc.scalar if fg == 3 else nc.sync
                eng.dma_start(
                    XF[:, fg, :],
                    bass.AP(tensor=x_in, offset=HOP * 128 * fg,
                            ap=[[HOP, 128], [1, FRAMELEN]]),
                )
            for fg in range(4):
                for b, eng in ((0, nc.sync), (1, nc.scalar)):
                    eng.dma_start(
                        bass.AP(tensor=dba.tensor,
                                offset=dba.offset + b * 4 * S + fg * S + HD,
                                ap=[list(dba.ap[0]), [1, S - HD]]),
                        bass.AP(tensor=d_in,
                                offset=b * SPAN + HOP * 128 * fg + WD + HD,
                                ap=[[HOP, 128], [1, S - HD]]),
                    )

            # repack e history (stride-8) into contiguous EOUT + DMA out,
            # chunked so the first chunk's DMA overlaps the loop tail
            def emit_eout(j0, j1):
                for g in range(8):
                    ea = ET[:]
                    src = bass.AP(tensor=ea.tensor,
                                  offset=ea.offset + 8 * (TSTART + j0) + g,
                                  ap=[list(ea.ap[0]), [8, j1 - j0]])
                    nc.vector.tensor_scalar_mul(EOUT[:, g, j0:j1], src,
                                                 1.0 / MU)
                eo = EOUT[:]
                for b, eng in ((0, nc.sync), (1, nc.scalar)):
                    eng.dma_start(
                        bass.AP(tensor=out_e, offset=b * F_LOC * TAIL + j0,
                                ap=[[TAIL, 128], [128 * TAIL, 4],
                                    [1, j1 - j0]]),
                        bass.AP(tensor=eo.tensor,
                                offset=eo.offset + b * 4 * TAIL + j0,
                                ap=[list(eo.ap[0]), [TAIL, 4], [1, j1 - j0]]),
                    )

            for t in range(S):
                i = t % NBUF
                cur, nxt = WALL[t % 2], WALL[(t + 1) % 2]
                tmp = TMP[t % 2]

                def emit_dot(g):
                    nc.vector.scalar_tensor_tensor(
                        out=PROD[i][:, g, :], in0=cur[:, g, :],
                        scalar=-MU, in1=XF[:, g % 4, t:t + K],
                        op0=AluOp.mult, op1=AluOp.mult,
                        accum_out=NS8[i][:, g:g + 1],
                    )

                def emit_e(h):
                    # e half: ET[:, t, 4h:4h+4] = d_t + ns, issued inside
                    # the dot stream so the accumulator-ack and TT-duration
                    # edges are covered by later dots' issue slots
                    da = DB[:]
                    dt = bass.AP(tensor=da.tensor,
                                 offset=da.offset + h * 4 * S + t,
                                 ap=[list(da.ap[0]), [S, 4]])
                    nc.vector.scalar_tensor_tensor(
                        out=ET[:, t, 4 * h:4 * h + 4], in0=dt, scalar=MU,
                        in1=NS8[i][:, 4 * h:4 * h + 4],
                        op0=AluOp.mult, op1=AluOp.add)

                # Order so every RAW edge keeps >=2 issue slots of
                # separation (RAW costs the predecessor's full duration,
                # 157 + FD/accel ns, + ~35 ack; TT256 issues in ~335).
                # Dots 4-7 first, eV halves tucked inside the dot stream,
                # then the h=1 pipeline leads so the h=0 twin covers each
                # RAW gap, and the step boundary ends ...clip_h1, clip_h0,
                # giving the scheduler two hoist slots for next-step dots
                # (4-7 after clip_h1, 0-3 after clip_h0).
                if t == 0:
                    # W=0: all dots are zero, e_0 = d_0 -- one copy replaces
                    # 8 dots + 2 e-merges and decouples step 0 from XF
                    da = DB[:]
                    d0 = bass.AP(tensor=da.tensor, offset=da.offset,
                                 ap=[list(da.ap[0]), [4 * S, B], [S, 4]])
                    nc.vector.tensor_scalar_mul(ET[:, 0, :], d0, MU)
                else:
                    for g in (4, 5, 6, 7, 0, 1):
                        emit_dot(g)
                    emit_e(1)       # needs dots 4-7
                    emit_dot(2)
                    emit_dot(3)
                    emit_e(0)       # needs dots 0-3
                if t == S - 1:
                    continue    # W after the last e is never used
                xa = XF[:]
                ea = ET[:]
                xh = bass.AP(tensor=xa.tensor, offset=xa.offset + t,
                             ap=[list(xa.ap[0]), [FRAMELEN, 4], [1, K]])
                for h in (1, 0):
                    ebc = bass.AP(tensor=ea.tensor,
                                  offset=ea.offset + 8 * t + 4 * h,
                                  ap=[list(ea.ap[0]), [1, 4], [0, K]])
                    nc.vector.tensor_tensor(out=tmp[:, 4 * h:4 * h + 4, :],
                                            in0=xh, in1=ebc, op=AluOp.mult)
                for h in (1, 0):
                    nc.vector.tensor_tensor(
                        out=nxt[:, 4 * h:4 * h + 4, :],
                        in0=cur[:, 4 * h:4 * h + 4, :],
                        in1=tmp[:, 4 * h:4 * h + 4, :], op=AluOp.add)
                for h in (1, 0):
                    nc.vector.tensor_scalar(
                        out=nxt[:, 4 * h:4 * h + 4, :],
                        in0=nxt[:, 4 * h:4 * h + 4, :],
                        scalar1=WMAX, scalar2=WMIN,
                        op0=AluOp.min, op1=AluOp.max,
                    )
                if t == TSTART + 128:
                    # first output chunk: its DMA overlaps the last steps
                    emit_eout(0, 128)

            emit_eout(128, TAIL)
            # head: frame 0 of this core (only core 0's matters), groups b*4
            eh = ET[:]
            for b in range(B):
                nc.sync.dma_start(
                    bass.AP(tensor=out_head, offset=b * TSTART,
                            ap=[[TSTART, 1], [1, TSTART]]),
                    bass.AP(tensor=eh.tensor, offset=eh.offset + 4 * b,
                            ap=[[eh.ap[0][0], 1], [8, TSTART]]),
                )
    nc.finalize()
    return nc


def _get_nc():
    if "nc" not in _CACHE:
        _CACHE["nc"] = _build()
    return _CACHE["nc"]


def run_shards(d, x, trace=False, **kw):
    from concourse.bass_utils import run_bass_kernel_spmd

    nc = _get_nc()
    in_maps = []
    for c in range(NC):
        lo = c * CORE_STRIDE
        in_maps.append({
            "x": np.ascontiguousarray(x[lo:lo + SPAN], dtype=np.float32),
            "d": np.ascontiguousarray(d[:, lo:lo + SPAN], dtype=np.float32),
        })
    return run_bass_kernel_spmd(nc, in_maps, core_ids=list(range(NC)),
                                trace=trace, **kw)


def assemble(results, d):
    es = np.stack([r["out_e"] for r in results])     # (8, B, 512, 192)
    head = results[0]["out_head"] * np.float32(1.0 / MU)  # (B, 224)

    # d windows: dwin[b, f, t] = d[b, 256 f + WD + t], t in [0, S)
    idx = HOP * np.arange(F)[:, None] + WD + np.arange(S)[None, :]
    dwin = d[:, idx]                                 # (B, F, S)

    def ola(head_v, main_v):
        # head_v: (B, TSTART) frame-0 steps t<TSTART
        # main_v: (B, F, TAIL) steps t in [TSTART, S) for every frame
        o = np.zeros((B, OUT_LEN), np.float32)
        o[:, WD:WD + TSTART] = head_v
        o[:, WD + TSTART:FRAMELEN - K] = main_v[:, 0]
        o[:, FRAMELEN - K:] = main_v[:, 1:].reshape(B, -1)
        return o

    e_main = es.transpose(1, 0, 2, 3).reshape(B, F, TAIL)
    e_out = ola(head, e_main)
    dest_out = ola(dwin[:, 0, :TSTART] - head,
                   dwin[:, :, TSTART:] - e_main)
    return dest_out, e_out


def kernel(d, x):
    d = np.asarray(d, dtype=np.float32)
    x = np.asarray(x, dtype=np.float32)
    res = run_shards(d, x)
    return assemble(res.results, d)
